# revision 1
# baseline (speedup 1.0000x reference)
"""Trainium2 Bass kernel for nn_ButterflyProduct.

Math: out = A_0 A_1 ... A_9 @ x_row for each batch row, where
A_i = sum_f softmax(logit)[i,f] * B_f and B_f is banded with offsets
{0, -d_f, +d_f}, d_f = 2^(9-f).  Each A_i therefore has 21 diagonals at
offsets {0, +-1, +-2, ..., +-512}.

On device (per core, batch sharded 8 ways):
  1. softmax(logit) -> prob (10,10), broadcast to all partitions.
  2. Compose T = A_0...A_9 (1024x1024, float32r) in 10 block-banded PE
     matmul steps starting from the identity.  The banded 128x128 lhsT
     blocks are materialized densely via a shear-DMA: band coefficients
     are written as columns of a (128, 256) table per block, staged to
     DRAM, and read back with a strided AP (row step = width-1) which
     lands each table column on a diagonal.
  3. U = T^T via PE transposes.
  4. out[b,:] = x[b,:] @ T^T: per 128-row batch tile, transpose x on the
     PE, then 16 accumulating f32r matmuls against U.

float32r runs the PE at full rate (1 cycle/row for N>=256) with ~1.5e-4
matmul relative error (vs 2.3e-3 for bf16).
"""

import sys

if "/opt/trn_rl_repo" not in sys.path:
    sys.path.insert(0, "/opt/trn_rl_repo")

import numpy as np

SIZE = 1024
MF = 10          # number of butterfly factors
NT = 10          # number of mixing terms
BATCH = 16384
N_CORES = 8
BPC = BATCH // N_CORES   # 2048 rows per core
NB = SIZE // 128         # 8 partition blocks
DIAG = [1 << (MF - 1 - f) for f in range(MF)]  # [512,256,128,64,32,16,8,4,2,1]
SMALL_D = [d for d in DIAG if d <= 64]         # [64,32,16,8,4,2,1]
F_OF_D = {DIAG[f]: f for f in range(MF)}
F128, F256, F512 = F_OF_D[128], F_OF_D[256], F_OF_D[512]

# (Delta, Mb) slots for the single-band blocks (d in {256, 512})
SINGLE_BLOCKS = (
    [(2, Mb) for Mb in range(6)]          # slots 0..5   coeff row 0 (S_256)
    + [(-2, Mb) for Mb in range(2, 8)]    # slots 6..11  coeff row 1 (Psh_256)
    + [(4, Mb) for Mb in range(4)]        # slots 12..15 coeff row 2 (S_512)
    + [(-4, Mb) for Mb in range(4, 8)]    # slots 16..19 coeff row 3 (Psh_512)
)
SINGLE_SLOT = {(dl, mb): s for s, (dl, mb) in enumerate(SINGLE_BLOCKS)}
SINGLE_COEFF_ROW = {2: 0, -2: 1, 4: 2, -4: 3}

_CACHE = {}


def _build_program():
    import concourse.bacc as bacc
    import concourse.bass as bass
    import concourse.mybir as mybir
    from concourse import tile

    F32 = mybir.dt.float32
    F32R = mybir.dt.float32r
    AX = mybir.AxisListType
    AF = mybir.ActivationFunctionType
    ALU = mybir.AluOpType

    nc = bacc.Bacc("TRN2", target_bir_lowering=False, debug=False)

    x_d = nc.dram_tensor("x", [BPC, SIZE], F32, kind="ExternalInput").ap()
    lg_d = nc.dram_tensor("logit", [NT, MF], F32, kind="ExternalInput").ap()
    dg_d = nc.dram_tensor("dgs", [128, MF, NB], F32, kind="ExternalInput").ap()
    sb_d = nc.dram_tensor("sbc", [128, MF, NB], F32, kind="ExternalInput").ap()
    sp_d = nc.dram_tensor("spsh", [128, MF, NB], F32, kind="ExternalInput").ap()
    id_d = nc.dram_tensor("idstrip", [128, 384], F32, kind="ExternalInput").ap()
    out_d = nc.dram_tensor("out", [BPC, SIZE], F32, kind="ExternalOutput").ap()
    pr_stage = nc.dram_tensor("pr_stage", [NT * MF], mybir.dt.float32).ap()
    # DRAM staging for the shear tables (double buffered)
    stages = {
        (s, p): nc.dram_tensor(f"stg_{s}{p}", [128, NB * 256], F32R).ap()
        for s in "CPM"
        for p in (0, 1)
    }

    def shear_src(s, p):
        """AP reading staged tables as dense banded blocks.

        block Mb, row m, col j  <-  stage[m, Mb*256 + 128 + j - m]
        flat offset = m*2048 + Mb*256 + 128 - m + j
        """
        flat = stages[(s, p)].rearrange("a b -> (a b)")
        return bass.AP(
            tensor=flat.tensor,
            offset=128,
            ap=[[NB * 256 - 1, 128], [256, NB], [1, 128]],
        )

    ncopy = [0]

    def rr_copy(out, in_):
        # round-robin PSUM->SBUF / SBUF->SBUF copies across DVE and ACT
        if ncopy[0] % 2 == 0:
            nc.vector.tensor_copy(out, in_)
        else:
            nc.scalar.copy(out, in_)
        ncopy[0] += 1

    with tile.TileContext(nc) as tc:
        with (
            tc.tile_pool(name="const", bufs=1) as cp,
            tc.tile_pool(name="coef", bufs=1) as kp,
            tc.tile_pool(name="T", bufs=1) as tp,
        ):
            # ---- load constants ----
            lg = cp.tile([NT, MF], F32, tag="lg")
            nc.sync.dma_start(lg[:, :], lg_d[:, :])
            dgs = cp.tile([128, MF, NB], F32, tag="dgs")
            nc.sync.dma_start(dgs[:, :, :], dg_d[:, :, :])
            sbc = cp.tile([128, MF, NB], F32, tag="sbc")
            nc.sync.dma_start(sbc[:, :, :], sb_d[:, :, :])
            spsh = cp.tile([128, MF, NB], F32, tag="spsh")
            nc.sync.dma_start(spsh[:, :, :], sp_d[:, :, :])
            idst = cp.tile([128, 384], F32, tag="idst")
            nc.sync.dma_start(idst[:, :], id_d[:, :])
            ident_r = cp.tile([128, 128], F32R, tag="identr")
            nc.vector.tensor_copy(ident_r[:, :], idst[:, 127:255])

            # ---- softmax(logit) ----
            mx = cp.tile([NT, 1], F32, tag="mx")
            nc.vector.reduce_max(mx[:, :], lg[:, :], axis=AX.X)
            lgs = cp.tile([NT, MF], F32, tag="lgs")
            nc.vector.tensor_scalar_sub(lgs[:, :], lg[:, :], mx[:, :])
            ex = cp.tile([NT, MF], F32, tag="ex")
            nc.scalar.activation(ex[:, :], lgs[:, :], AF.Exp)
            sm = cp.tile([NT, 1], F32, tag="sm")
            nc.vector.reduce_sum(sm[:, :], ex[:, :], axis=AX.X)
            rs = cp.tile([NT, 1], F32, tag="rs")
            nc.vector.reciprocal(rs[:, :], sm[:, :])
            pr = cp.tile([NT, MF], F32, tag="pr")
            nc.vector.tensor_scalar_mul(pr[:, :], ex[:, :], rs[:, :])
            # broadcast probs to all 128 partitions (bounce via DRAM to get
            # a single-partition flat row first; partition_broadcast needs p0)
            nc.sync.dma_start(pr_stage.rearrange("(a b) -> a b", a=NT, b=MF), pr[:, :])
            prf = cp.tile([1, NT * MF], F32, tag="prf")
            nc.sync.dma_start(prf[:, :], pr_stage[None, :])
            pbc = cp.tile([128, NT, MF], F32, tag="pbc")
            nc.gpsimd.partition_broadcast(
                pbc[:, :, :].rearrange("p a b -> p (a b)"), prf[:, :]
            )

            # ---- T ping-pong buffers, T <- I ----
            Ta = [tp.tile([128, SIZE], F32R, tag=f"Ta{J}", name=f"Ta{J}") for J in range(NB)]
            Tb = [tp.tile([128, SIZE], F32R, tag=f"Tb{J}", name=f"Tb{J}") for J in range(NB)]
            for J in range(NB):
                nc.vector.memset(Ta[J][:, :].bitcast(F32), 0.0)
                nc.vector.tensor_copy(
                    Ta[J][:, 128 * J : 128 * J + 128], idst[:, 127:255]
                )

            # ---- compose: 10 steps of T <- A_i @ T ----
            with (
                tc.tile_pool(name="tab", bufs=1) as tbp,
                tc.tile_pool(name="lhs", bufs=1) as lp,
                tc.tile_pool(name="ctmp", bufs=2) as ctp,
                tc.tile_pool(name="cps", bufs=4, space="PSUM") as cps,
            ):
                tabs = {
                    (s, p): tbp.tile([128, NB, 256], F32R, tag=f"tab{s}{p}", name=f"tab{s}{p}")
                    for s in "CPM"
                    for p in (0, 1)
                }
                for t in tabs.values():
                    nc.vector.memset(t[:, :, :].bitcast(F32), 0.0)
                lhs = {
                    (s, p): lp.tile([128, NB, 128], F32R, tag=f"lhs{s}{p}", name=f"lhs{s}{p}")
                    for s in "CPM"
                    for p in (0, 1)
                }
                lhsS = {
                    p: lp.tile([128, 20, 128], F32R, tag=f"lhsS{p}", name=f"lhsS{p}") for p in (0, 1)
                }

                cur, nxt = Ta, Tb
                for st, i in enumerate(reversed(range(NT))):
                    p = st % 2
                    tC, tP, tM = tabs[("C", p)], tabs[("P", p)], tabs[("M", p)]

                    def pcol(f, i=i):
                        return pbc[:, i, f : f + 1]

                    # D band: accumulate sum_f p_if * dg_f with fused MACs
                    dtmp = ctp.tile([128, NB], F32, tag="dtmp")
                    nc.vector.tensor_scalar_mul(dtmp[:, :], dgs[:, 0, :], pcol(0))
                    for f in range(1, MF):
                        nc.vector.scalar_tensor_tensor(
                            dtmp[:, :], dgs[:, f, :], pcol(f), dtmp[:, :],
                            op0=ALU.mult, op1=ALU.add,
                        )
                    nc.vector.tensor_copy(tC[:, :, 128], dtmp[:, :])

                    # banded columns (each: one tensor_scalar over 8 strided cols)
                    for d in SMALL_D:
                        f = F_OF_D[d]
                        nc.vector.tensor_scalar_mul(tC[:, :, 128 + d], sbc[:, f, :], pcol(f))
                        nc.vector.tensor_scalar_mul(tC[:, :, 128 - d], spsh[:, f, :], pcol(f))
                        nc.scalar.activation(tP[:, :, d], sbc[:, f, :], AF.Copy, scale=pcol(f))
                        nc.scalar.activation(tM[:, :, 256 - d], spsh[:, f, :], AF.Copy, scale=pcol(f))
                    nc.scalar.activation(tP[:, :, 128], sbc[:, F128, :], AF.Copy, scale=pcol(F128))
                    nc.scalar.activation(tM[:, :, 128], spsh[:, F128, :], AF.Copy, scale=pcol(F128))

                    # stage + shear-read back as dense blocks
                    for s in "CPM":
                        nc.sync.dma_start(
                            stages[(s, p)][:, :],
                            tabs[(s, p)][:, :, :].rearrange("a b c -> a (b c)"),
                        )
                        nc.sync.dma_start(lhs[(s, p)][:, :, :], shear_src(s, p))

                    # single-band blocks (d in {256,512}) via shifted-identity
                    s4 = ctp.tile([128, 4, NB], F32, tag="s4")
                    nc.scalar.activation(s4[:, 0, :], sbc[:, F256, :], AF.Copy, scale=pcol(F256))
                    nc.scalar.activation(s4[:, 1, :], spsh[:, F256, :], AF.Copy, scale=pcol(F256))
                    nc.scalar.activation(s4[:, 2, :], sbc[:, F512, :], AF.Copy, scale=pcol(F512))
                    nc.scalar.activation(s4[:, 3, :], spsh[:, F512, :], AF.Copy, scale=pcol(F512))
                    for slot, (dl, Mb) in enumerate(SINGLE_BLOCKS):
                        crow = SINGLE_COEFF_ROW[dl]
                        nc.vector.tensor_scalar_mul(
                            lhsS[p][:, slot, :], idst[:, 127:255],
                            s4[:, crow, Mb : Mb + 1],
                        )

                    # block-banded matmuls: T_next[J] = sum_M lhsT(M,J).T @ T[M]
                    for Jb in range(NB):
                        mms = []
                        if True:
                            mms.append(lhs[("C", p)][:, Jb, :])          # Delta 0
                        if Jb >= 1:
                            mms.append(lhs[("P", p)][:, Jb - 1, :])      # Delta +1
                        if Jb <= 6:
                            mms.append(lhs[("M", p)][:, Jb + 1, :])      # Delta -1
                        for dl in (2, -2, 4, -4):
                            Mb = Jb - dl
                            if 0 <= Mb < NB:
                                mms.append(lhsS[p][:, SINGLE_SLOT[(dl, Mb)], :])
                        mbs = []
                        if True:
                            mbs.append(Jb)
                        if Jb >= 1:
                            mbs.append(Jb - 1)
                        if Jb <= 6:
                            mbs.append(Jb + 1)
                        for dl in (2, -2, 4, -4):
                            Mb = Jb - dl
                            if 0 <= Mb < NB:
                                mbs.append(Mb)
                        for h in range(2):
                            ps = cps.tile([128, 512], F32, tag="cacc")
                            for idx, (lh, Mb) in enumerate(zip(mms, mbs)):
                                nc.tensor.matmul(
                                    ps[:, :], lh,
                                    cur[Mb][:, 512 * h : 512 * h + 512],
                                    start=(idx == 0), stop=(idx == len(mms) - 1),
                                )
                            rr_copy(nxt[Jb][:, 512 * h : 512 * h + 512], ps[:, :])
                    cur, nxt = nxt, cur

            # ---- U = T^T ----
            T_fin = cur
            U = [tp.tile([128, SIZE], F32R, tag=f"U{K}", name=f"U{K}") for K in range(NB)]
            with tc.tile_pool(name="tps", bufs=4, space="PSUM") as tps:
                for Jb in range(NB):
                    for Kb in range(NB):
                        pt = tps.tile([128, 128], F32R, tag="tp")
                        nc.tensor.transpose(
                            pt[:, :], T_fin[Jb][:, 128 * Kb : 128 * Kb + 128],
                            ident_r[:, :],
                        )
                        rr_copy(U[Kb][:, 128 * Jb : 128 * Jb + 128], pt[:, :])

            # ---- batch phase ----
            with (
                tc.tile_pool(name="xin", bufs=3) as xin,
                tc.tile_pool(name="xt", bufs=3) as xtp,
                tc.tile_pool(name="op", bufs=3) as op,
                tc.tile_pool(name="xps", bufs=4, space="PSUM") as xps,
                tc.tile_pool(name="ops", bufs=4, space="PSUM") as ops,
            ):
                for t in range(BPC // 128):
                    xi = xin.tile([128, SIZE], F32, tag="xi")
                    nc.sync.dma_start(xi[:, :], x_d[128 * t : 128 * t + 128, :])
                    xt = xtp.tile([128, SIZE], F32R, tag="xt")
                    for k in range(NB):
                        tpx = xps.tile([128, 128], F32, tag="tpx")
                        nc.tensor.transpose(
                            tpx[:, :], xi[:, 128 * k : 128 * k + 128],
                            idst[:, 127:255],
                        )
                        rr_copy(xt[:, 128 * k : 128 * k + 128], tpx[:, :])
                    ob = op.tile([128, SIZE], F32, tag="ob")
                    for h in range(2):
                        ps = ops.tile([128, 512], F32, tag="oacc")
                        for k in range(NB):
                            nc.tensor.matmul(
                                ps[:, :], xt[:, 128 * k : 128 * k + 128],
                                U[k][:, 512 * h : 512 * h + 512],
                                start=(k == 0), stop=(k == NB - 1),
                            )
                        rr_copy(ob[:, 512 * h : 512 * h + 512], ps[:, :])
                    nc.sync.dma_start(out_d[128 * t : 128 * t + 128, :], ob[:, :])

    nc.compile()
    return nc


def _get_program():
    if "nc" not in _CACHE:
        _CACHE["nc"] = _build_program()
    return _CACHE["nc"]


LAST_RESULTS = {}


def kernel(input, diags, subdiags, superdiags, logit, _trace=False):
    from concourse.bass_utils import run_bass_kernel_spmd

    x = np.ascontiguousarray(np.asarray(input, dtype=np.float32))
    dg = np.asarray(diags, dtype=np.float32)
    sb = np.asarray(subdiags, dtype=np.float32)
    sp = np.asarray(superdiags, dtype=np.float32)
    lg = np.ascontiguousarray(np.asarray(logit, dtype=np.float32))

    # host staging (pure layout): zero unused tails, shift superdiags by d,
    # relayout coefficient vectors partition-major (m, f, block)
    sb_clean = np.zeros_like(sb)
    sp_shift = np.zeros_like(sp)
    for f in range(MF):
        d = DIAG[f]
        sb_clean[f, : SIZE - d] = sb[f, : SIZE - d]
        sp_shift[f, d:] = sp[f, : SIZE - d]

    def pm(v):  # (MF, SIZE) -> (128, MF, NB) with [m, f, blk] = v[f, 128*blk + m]
        return np.ascontiguousarray(
            v.reshape(MF, NB, 128).transpose(2, 0, 1)
        )

    dgs = pm(dg)
    sbc = pm(sb_clean)
    spsh = pm(sp_shift)
    idstrip = np.zeros((128, 384), dtype=np.float32)
    for m in range(128):
        idstrip[m, m + 127] = 1.0

    nc = _get_program()
    in_maps = []
    for c in range(N_CORES):
        in_maps.append(
            {
                "x": x[BPC * c : BPC * (c + 1)],
                "logit": lg,
                "dgs": dgs,
                "sbc": sbc,
                "spsh": spsh,
                "idstrip": idstrip,
            }
        )
    res = run_bass_kernel_spmd(nc, in_maps, core_ids=list(range(N_CORES)), trace=_trace)
    LAST_RESULTS["res"] = res
    out = np.concatenate([res.results[c]["out"] for c in range(N_CORES)], axis=0)
    return out



# revision 6
# speedup vs baseline: 1.0543x; 1.0543x over previous
"""Trainium2 Bass kernel for nn_ButterflyProduct.

Math: out = A_0 A_1 ... A_9 @ x_row for each batch row, where
A_i = sum_f softmax(logit)[i,f] * B_f and B_f is banded with offsets
{0, -d_f, +d_f}, d_f = 2^(9-f).  Each A_i therefore has 21 diagonals at
offsets {0, +-1, +-2, ..., +-512}.

Per core (batch sharded 8 ways), all compute in bf16 on the PE where
possible:
  1. softmax(logit) -> prob (10,10), broadcast to all partitions.
  2. Compose T = A_0...A_9 (1024x1024 bf16) in 10 steps T <- A_i @ T.
     Block structure of A_i (128-blocks, Delta = block col - block row):
       Delta 0   : 15-diagonal banded block       -> PE matmul (C)
       Delta +-1 : d=128 diagonal + d<=64 corner  -> PE matmul (P/M)
       Delta +-2 : d=256 pure diagonal block      -> vector MAC
       Delta +-4 : d=512 pure diagonal block      -> vector MAC
     The C/P/M banded stationaries are materialized densely via a
     shear-DMA (coefficients written as columns of a narrow window per
     block, staged to DRAM, read back with a strided AP that lands each
     column on a diagonal).  The pure-diagonal blocks never touch the
     PE: they are per-partition scalar_tensor_tensor MACs on DVE/Pool
     reading the PE's PSUM partial.
     Step 1 (T = A_9 @ I) is specialized to 128-col windows.
  3. U = T^T via DMA-crossbar transposes (16x128 xbar tiles).
  4. out[b,:] = x[b,:] @ T^T: x is uploaded pre-cast to bf16 and
     DMA-crossbar-transposed straight into stationary layout; 16
     accumulating bf16 matmuls per 128-row tile.
"""

import sys

if "/opt/trn_rl_repo" not in sys.path:
    sys.path.insert(0, "/opt/trn_rl_repo")

import numpy as np

SIZE = 1024
MF = 10          # number of butterfly factors
NT = 10          # number of mixing terms
BATCH = 16384
N_CORES = 8
BPC = BATCH // N_CORES   # 2048 rows per core
NB = SIZE // 128         # 8 partition blocks
DIAG = [1 << (MF - 1 - f) for f in range(MF)]  # [512,256,128,64,32,16,8,4,2,1]
SMALL_D = [d for d in DIAG if d <= 64]         # [64,32,16,8,4,2,1]
F_OF_D = {DIAG[f]: f for f in range(MF)}
F128, F256, F512 = F_OF_D[128], F_OF_D[256], F_OF_D[512]

# per-table live-column window (start, width) in 256-wide table coords.
# S/Z are the pure-diagonal d=512 blocks (Delta -4/+4): single live col 128.
WOFF = {"C": 64, "P": 0, "M": 124, "S": 128, "Z": 128}
TW = {"C": 132, "P": 132, "M": 132, "S": 4, "Z": 4}
# vector-MAC singles: only the d=256 diagonal blocks (Delta -2/+2).
# For output block Jb the source block is Mb = Jb + delta; coeff column
# is s4[:, row, Mb].
SINGLES = [(0, -2), (1, 2)]

_CACHE = {}


def _build_program():
    import concourse.bacc as bacc
    import concourse.bass as bass
    import concourse.mybir as mybir
    from concourse import tile

    F32 = mybir.dt.float32
    BF16 = mybir.dt.bfloat16
    AX = mybir.AxisListType
    AF = mybir.ActivationFunctionType
    ALU = mybir.AluOpType

    nc = bacc.Bacc("TRN2", target_bir_lowering=False, debug=False)

    xb_d = nc.dram_tensor("xb", [BPC, SIZE], BF16, kind="ExternalInput").ap()
    lg_d = nc.dram_tensor("logit", [NT, MF], F32, kind="ExternalInput").ap()
    dg_d = nc.dram_tensor("dgs", [128, MF, NB], F32, kind="ExternalInput").ap()
    sb_d = nc.dram_tensor("sbc", [128, MF, NB], F32, kind="ExternalInput").ap()
    sp_d = nc.dram_tensor("spsh", [128, MF, NB], F32, kind="ExternalInput").ap()
    id_d = nc.dram_tensor("idstrip", [128, 384], BF16, kind="ExternalInput").ap()
    out_d = nc.dram_tensor("out", [BPC, SIZE], F32, kind="ExternalOutput").ap()
    pr_stage = nc.dram_tensor("pr_stage", [NT * MF], F32).ap()
    # DRAM staging for the shear tables (double buffered)
    stages = {
        (s, p): nc.dram_tensor(f"stg_{s}{p}", [128, NB * 256], BF16).ap()
        for s in "CPMSZ"
        for p in (0, 1)
    }

    def shear_src(s, p):
        """AP reading staged tables as dense banded blocks.

        block Mb, row m, col j  <-  stage[m, Mb*256 + 128 + j - m]
        flat offset = m*2048 + Mb*256 + 128 - m + j
        """
        flat = stages[(s, p)].rearrange("a b -> (a b)")
        return bass.AP(
            tensor=flat.tensor,
            offset=128,
            ap=[[NB * 256 - 1, 128], [256, NB], [1, 128]],
        )

    def mac_engine():
        # Pool has no ALU/PSUM path on TRN2; all MACs go to DVE
        return nc.vector

    with tile.TileContext(nc) as tc:
        with (
            tc.tile_pool(name="const", bufs=1) as cp,
            tc.tile_pool(name="T", bufs=1) as tp,
        ):
            # ---- load constants ----
            lg = cp.tile([NT, MF], F32, tag="lg")
            nc.sync.dma_start(lg[:, :], lg_d[:, :])
            dgs = cp.tile([128, MF, NB], F32, tag="dgs")
            nc.sync.dma_start(dgs[:, :, :], dg_d[:, :, :])
            sbc = cp.tile([128, MF, NB], F32, tag="sbc")
            nc.sync.dma_start(sbc[:, :, :], sb_d[:, :, :])
            spsh = cp.tile([128, MF, NB], F32, tag="spsh")
            nc.sync.dma_start(spsh[:, :, :], sp_d[:, :, :])
            idst = cp.tile([128, 384], BF16, tag="idst")
            nc.sync.dma_start(idst[:, :], id_d[:, :])

            # zero strip for stage-table padding (bf16 zeros)
            ztile = cp.tile([128, NB * 256], BF16, tag="ztile")
            nc.vector.memset(ztile[:, :], 0.0)
            for key in stages:
                nc.sync.dma_start(stages[key][:, :], ztile[:, :])

            # ---- softmax(logit) ----
            mx = cp.tile([NT, 1], F32, tag="mx")
            nc.vector.reduce_max(mx[:, :], lg[:, :], axis=AX.X)
            lgs = cp.tile([NT, MF], F32, tag="lgs")
            nc.vector.tensor_scalar_sub(lgs[:, :], lg[:, :], mx[:, :])
            ex = cp.tile([NT, MF], F32, tag="ex")
            nc.scalar.activation(ex[:, :], lgs[:, :], AF.Exp)
            sm = cp.tile([NT, 1], F32, tag="sm")
            nc.vector.reduce_sum(sm[:, :], ex[:, :], axis=AX.X)
            rs = cp.tile([NT, 1], F32, tag="rs")
            nc.vector.reciprocal(rs[:, :], sm[:, :])
            pr = cp.tile([NT, MF], F32, tag="pr")
            nc.vector.tensor_scalar_mul(pr[:, :], ex[:, :], rs[:, :])
            # broadcast probs to all 128 partitions (bounce via DRAM to get
            # a single-partition flat row first; partition_broadcast needs p0)
            nc.sync.dma_start(pr_stage.rearrange("(a b) -> a b", a=NT, b=MF), pr[:, :])
            prf = cp.tile([1, NT * MF], F32, tag="prf")
            nc.sync.dma_start(prf[:, :], pr_stage[None, :])
            pbc = cp.tile([128, NT, MF], F32, tag="pbc")
            nc.gpsimd.partition_broadcast(
                pbc[:, :, :].rearrange("p a b -> p (a b)"), prf[:, :]
            )

            # ---- T ping-pong buffers (bf16), T <- I ----
            Ta = [tp.tile([128, SIZE], BF16, tag=f"Ta{J}", name=f"Ta{J}") for J in range(NB)]
            Tb = [tp.tile([128, SIZE], BF16, tag=f"Tb{J}", name=f"Tb{J}") for J in range(NB)]
            for J in range(NB):
                nc.vector.memset(Ta[J][:, :], 0.0)
                nc.vector.tensor_copy(
                    Ta[J][:, 128 * J : 128 * J + 128], idst[:, 127:255]
                )

            # U = T^T, one contiguous tensor: U[p, k, n] = T[n, 128k+p]
            U = tp.tile([128, NB, SIZE], BF16, tag="U", name="U")

            # ---- compose: 10 steps of T <- A_i @ T ----
            with (
                tc.tile_pool(name="tab", bufs=1) as tbp,
                tc.tile_pool(name="lhs", bufs=1) as lp,
                tc.tile_pool(name="ctmp", bufs=2) as ctp,
                tc.tile_pool(name="cps", bufs=6, space="PSUM") as cps,
            ):
                # live-window tables in SBUF (fixed tiles, zero padding
                # written once; only live columns rewritten per step)
                tabs = {
                    (s, p): tbp.tile([128, NB, TW[s]], BF16, tag=f"tab{s}{p}", name=f"tab{s}{p}")
                    for s in "CPMSZ"
                    for p in (0, 1)
                }
                for t in tabs.values():
                    nc.vector.memset(t[:, :, :], 0.0)
                lhs = {
                    (s, p): lp.tile([128, NB, 128], BF16, tag=f"lhs{s}{p}", name=f"lhs{s}{p}")
                    for s in "CPMSZ"
                    for p in (0, 1)
                }
                s4s = {
                    p: lp.tile([128, 2, NB], F32, tag=f"s4_{p}", name=f"s4_{p}")
                    for p in (0, 1)
                }

                # PE warmup: keep the tensor engine busy while the first
                # tables build + stage, so it is at full p-state when the
                # real matmuls start.
                wps = cps.tile([128, 512], F32, tag="cacc")
                for w in range(10):
                    nc.tensor.matmul(
                        wps[:, :], idst[:, 127:255], Ta[0][:, 0:512],
                        start=(w == 0), stop=(w == 9),
                    )

                def build_tables(st, i):
                    """Vector-engine writes of step i's live table columns
                    + scaled singles coefficients (buffer p = st%2)."""
                    p = st % 2
                    tC, tP, tM = tabs[("C", p)], tabs[("P", p)], tabs[("M", p)]

                    def pcol(f):
                        return pbc[:, i, f : f + 1]

                    # D band: accumulate sum_f p_if * dg_f with fused MACs
                    dtmp = ctp.tile([128, NB], F32, tag="dtmp")
                    nc.vector.tensor_scalar_mul(dtmp[:, :], dgs[:, 0, :], pcol(0))
                    for f in range(1, MF - 1):
                        nc.vector.scalar_tensor_tensor(
                            dtmp[:, :], dgs[:, f, :], pcol(f), dtmp[:, :],
                            op0=ALU.mult, op1=ALU.add,
                        )
                    # last MAC writes straight into the C window (bf16)
                    nc.vector.scalar_tensor_tensor(
                        tC[:, :, 128 - WOFF["C"]], dgs[:, MF - 1, :], pcol(MF - 1),
                        dtmp[:, :], op0=ALU.mult, op1=ALU.add,
                    )

                    # banded columns (each: one op over 8 strided cols)
                    for n, d in enumerate(SMALL_D):
                        f = F_OF_D[d]
                        eng = nc.vector if n % 2 == 0 else nc.gpsimd
                        eng.tensor_scalar_mul(
                            tC[:, :, 128 + d - WOFF["C"]], sbc[:, f, :], pcol(f))
                        eng.tensor_scalar_mul(
                            tC[:, :, 128 - d - WOFF["C"]], spsh[:, f, :], pcol(f))
                        nc.scalar.activation(
                            tP[:, :, d - WOFF["P"]], sbc[:, f, :], AF.Copy, scale=pcol(f))
                        nc.scalar.activation(
                            tM[:, :, 256 - d - WOFF["M"]], spsh[:, f, :], AF.Copy, scale=pcol(f))
                    nc.scalar.activation(
                        tP[:, :, 128 - WOFF["P"]], sbc[:, F128, :], AF.Copy, scale=pcol(F128))
                    nc.scalar.activation(
                        tM[:, :, 128 - WOFF["M"]], spsh[:, F128, :], AF.Copy, scale=pcol(F128))

                    # d=512 pure-diagonal stationaries (tables S/Z, col 128)
                    tS, tZ = tabs[("S", p)], tabs[("Z", p)]
                    nc.scalar.activation(tS[:, :, 128 - WOFF["S"]], sbc[:, F512, :], AF.Copy, scale=pcol(F512))
                    nc.scalar.activation(tZ[:, :, 128 - WOFF["Z"]], spsh[:, F512, :], AF.Copy, scale=pcol(F512))

                    # d=256 singles coefficients, scaled by p_if
                    s4 = s4s[p]
                    nc.scalar.activation(s4[:, 0, :], sbc[:, F256, :], AF.Copy, scale=pcol(F256))
                    nc.scalar.activation(s4[:, 1, :], spsh[:, F256, :], AF.Copy, scale=pcol(F256))

                    # stage live windows + shear-read back as dense blocks
                    for s in "CPMSZ":
                        stg = stages[(s, p)].rearrange("a (b c) -> a b c", b=NB, c=256)
                        nc.sync.dma_start(
                            stg[:, :, WOFF[s] : WOFF[s] + TW[s]],
                            tabs[(s, p)][:, :, :],
                        )
                        nc.sync.dma_start(lhs[(s, p)][:, :, :], shear_src(s, p))

                # prefetch tables for the first two steps
                order = list(reversed(range(NT)))
                build_tables(0, order[0])
                build_tables(1, order[1])

                cur, nxt = Ta, Tb
                for st, i in enumerate(order):
                    p = st % 2
                    lC, lP, lM = lhs[("C", p)], lhs[("P", p)], lhs[("M", p)]
                    lS, lZ = lhs[("S", p)], lhs[("Z", p)]
                    s4 = s4s[p]

                    if st == 0:
                        # T = A_i @ I: everything restricted to 128-col
                        # aligned windows; column block Mb of nxt[Jb] is
                        # A_i's (Jb, Mb) block.
                        for Jb in range(NB):
                            live = []
                            for Mb in (Jb, Jb - 1, Jb + 1, Jb - 2, Jb + 2,
                                       Jb - 4, Jb + 4):
                                if 0 <= Mb < NB:
                                    live.append(Mb)
                            # PE handles d=512 diagonal blocks too
                            s1_extra = []
                            if Jb >= 4:
                                s1_extra.append((lS, Jb - 4))
                            if Jb <= NB - 5:
                                s1_extra.append((lZ, Jb + 4))
                            # zero the dead column blocks
                            for Mb in range(NB):
                                if Mb not in live:
                                    eng = nc.vector if Mb % 2 == 0 else nc.gpsimd
                                    eng.memset(
                                        nxt[Jb][:, 128 * Mb : 128 * Mb + 128], 0.0)
                            # PE blocks
                            pe_blocks = [(lC, Jb)]
                            if Jb >= 1:
                                pe_blocks.append((lP, Jb - 1))
                            if Jb <= NB - 2:
                                pe_blocks.append((lM, Jb + 1))
                            pe_blocks += s1_extra
                            for lt, Mb in pe_blocks:
                                lh = lt[:, Mb, :]
                                ps = cps.tile([128, 512], F32, tag="cacc")
                                nc.tensor.matmul(
                                    ps[:, 0:128], lh,
                                    cur[Mb][:, 128 * Mb : 128 * Mb + 128],
                                    start=True, stop=True,
                                )
                                nc.scalar.copy(
                                    nxt[Jb][:, 128 * Mb : 128 * Mb + 128],
                                    ps[:, 0:128])
                            # pure-diagonal blocks: coeff * I-block
                            for row, dl in SINGLES:
                                Mb = Jb + dl
                                if not (0 <= Mb < NB):
                                    continue
                                mac_engine().tensor_scalar_mul(
                                    nxt[Jb][:, 128 * Mb : 128 * Mb + 128],
                                    cur[Mb][:, 128 * Mb : 128 * Mb + 128],
                                    s4[:, row, Mb : Mb + 1],
                                )
                    else:
                        for Jb in range(NB):
                            mms = [(lC[:, Jb, :], Jb)]
                            if Jb >= 1:
                                mms.append((lP[:, Jb - 1, :], Jb - 1))
                            if Jb <= NB - 2:
                                mms.append((lM[:, Jb + 1, :], Jb + 1))
                            if Jb >= 4:
                                mms.append((lS[:, Jb - 4, :], Jb - 4))
                            if Jb <= NB - 5:
                                mms.append((lZ[:, Jb + 4, :], Jb + 4))
                            sgl = [
                                (row, Jb + dl) for row, dl in SINGLES
                                if 0 <= Jb + dl < NB
                            ]
                            for h in range(2):
                                hs = slice(512 * h, 512 * h + 512)
                                ps = cps.tile([128, 512], F32, tag="cacc")
                                for idx, (lh, Mb) in enumerate(mms):
                                    nc.tensor.matmul(
                                        ps[:, :], lh, cur[Mb][:, hs],
                                        start=(idx == 0),
                                        stop=(idx == len(mms) - 1),
                                    )
                                # PSUM partial -> bf16 accumulator in nxt
                                nc.scalar.copy(nxt[Jb][:, hs], ps[:, :])
                                # pure-diagonal MACs on top (in place)
                                for row, Mb in sgl:
                                    mac_engine().scalar_tensor_tensor(
                                        nxt[Jb][:, hs], cur[Mb][:, hs],
                                        s4[:, row, Mb : Mb + 1], nxt[Jb][:, hs],
                                        op0=ALU.mult, op1=ALU.add,
                                    )
                    if st + 2 < NT:
                        build_tables(st + 2, order[st + 2])
                    cur, nxt = nxt, cur

                # ---- U = T^T via DMA crossbar transposes ----
                T_fin = cur
                for Jb in range(NB):
                    nc.sync.dma_start(
                        U[:, :, 128 * Jb : 128 * Jb + 128],
                        T_fin[Jb][:, :],
                        transpose=True,
                    )

            # ---- batch phase ----
            with (
                tc.tile_pool(name="xt", bufs=16) as xtp,
                tc.tile_pool(name="op", bufs=4) as op,
                tc.tile_pool(name="ops", bufs=4, space="PSUM") as ops,
            ):
                # x arrives bf16; DMA-crossbar transpose straight into
                # stationary layout: xt[p, k, b] = x[b, 128k+p]
                xts = []
                for t in range(BPC // 128):
                    xt = xtp.tile([128, NB, 128], BF16, tag="xt")
                    nc.sync.dma_start(
                        xt[:, :, :],
                        xb_d[128 * t : 128 * t + 128, :],
                        transpose=True,
                    )
                    xts.append(xt)
                nout = [0]
                for h in range(2):
                    hs = slice(512 * h, 512 * h + 512)
                    for t in range(BPC // 128):
                        ps = ops.tile([128, 512], F32, tag="oacc")
                        for k in range(NB):
                            nc.tensor.matmul(
                                ps[:, :], xts[t][:, k, :], U[:, k, hs],
                                start=(k == 0), stop=(k == NB - 1),
                            )
                        ob = op.tile([128, 512], F32, tag="ob")
                        if nout[0] % 2 == 0:
                            nc.scalar.copy(ob[:, :], ps[:, :])
                        else:
                            nc.vector.tensor_copy(ob[:, :], ps[:, :])
                        nout[0] += 1
                        nc.sync.dma_start(
                            out_d[128 * t : 128 * t + 128, hs], ob[:, :])

    nc.compile()
    return nc


def _get_program():
    if "nc" not in _CACHE:
        _CACHE["nc"] = _build_program()
    return _CACHE["nc"]


LAST_RESULTS = {}


def kernel(input, diags, subdiags, superdiags, logit, _trace=False):
    import ml_dtypes
    from concourse.bass_utils import run_bass_kernel_spmd

    x = np.ascontiguousarray(np.asarray(input, dtype=np.float32))
    dg = np.asarray(diags, dtype=np.float32)
    sb = np.asarray(subdiags, dtype=np.float32)
    sp = np.asarray(superdiags, dtype=np.float32)
    lg = np.ascontiguousarray(np.asarray(logit, dtype=np.float32))

    xb = x.astype(ml_dtypes.bfloat16)

    # host staging (pure layout): zero unused tails, shift superdiags by d,
    # relayout coefficient vectors partition-major (m, f, block)
    sb_clean = np.zeros_like(sb)
    sp_shift = np.zeros_like(sp)
    for f in range(MF):
        d = DIAG[f]
        sb_clean[f, : SIZE - d] = sb[f, : SIZE - d]
        sp_shift[f, d:] = sp[f, : SIZE - d]

    def pm(v):  # (MF, SIZE) -> (128, MF, NB) with [m, f, blk] = v[f, 128*blk + m]
        return np.ascontiguousarray(
            v.reshape(MF, NB, 128).transpose(2, 0, 1)
        )

    dgs = pm(dg)
    sbc = pm(sb_clean)
    spsh = pm(sp_shift)
    idstrip = np.zeros((128, 384), dtype=np.float32)
    for m in range(128):
        idstrip[m, m + 127] = 1.0
    idstrip = idstrip.astype(ml_dtypes.bfloat16)

    nc = _get_program()
    in_maps = []
    for c in range(N_CORES):
        in_maps.append(
            {
                "xb": xb[BPC * c : BPC * (c + 1)],
                "logit": lg,
                "dgs": dgs,
                "sbc": sbc,
                "spsh": spsh,
                "idstrip": idstrip,
            }
        )
    res = run_bass_kernel_spmd(nc, in_maps, core_ids=list(range(N_CORES)), trace=_trace)
    LAST_RESULTS["res"] = res
    out = np.concatenate([res.results[c]["out"] for c in range(N_CORES)], axis=0)
    return out


# revision 7
# speedup vs baseline: 1.0725x; 1.0172x over previous
"""Trainium2 Bass kernel for nn_ButterflyProduct.

Math: out = A_0 A_1 ... A_9 @ x_row for each batch row, where
A_i = sum_f softmax(logit)[i,f] * B_f and B_f is banded with offsets
{0, -d_f, +d_f}, d_f = 2^(9-f).  Each A_i therefore has 21 diagonals at
offsets {0, +-1, +-2, ..., +-512}.

Per core (batch sharded 8 ways), all compute in bf16 on the PE where
possible:
  1. softmax(logit) -> prob (10,10), broadcast to all partitions.
  2. Compose T = A_0...A_9 (1024x1024 bf16) in 10 steps T <- A_i @ T.
     Block structure of A_i (128-blocks, Delta = block col - block row):
       Delta 0   : 15-diagonal banded block       -> PE matmul (C)
       Delta +-1 : d=128 diagonal + d<=64 corner  -> PE matmul (P/M)
       Delta +-2 : d=256 pure diagonal block      -> vector MAC
       Delta +-4 : d=512 pure diagonal block      -> vector MAC
     The C/P/M banded stationaries are materialized densely via a
     shear-DMA (coefficients written as columns of a narrow window per
     block, staged to DRAM, read back with a strided AP that lands each
     column on a diagonal).  The pure-diagonal blocks never touch the
     PE: they are per-partition scalar_tensor_tensor MACs on DVE/Pool
     reading the PE's PSUM partial.
     Step 1 (T = A_9 @ I) is specialized to 128-col windows.
  3. U = T^T via DMA-crossbar transposes (16x128 xbar tiles).
  4. out[b,:] = x[b,:] @ T^T: x is uploaded pre-cast to bf16 and
     DMA-crossbar-transposed straight into stationary layout; 16
     accumulating bf16 matmuls per 128-row tile.
"""

import sys

if "/opt/trn_rl_repo" not in sys.path:
    sys.path.insert(0, "/opt/trn_rl_repo")

import numpy as np

SIZE = 1024
MF = 10          # number of butterfly factors
NT = 10          # number of mixing terms
BATCH = 16384
N_CORES = 8
BPC = BATCH // N_CORES   # 2048 rows per core
NB = SIZE // 128         # 8 partition blocks
DIAG = [1 << (MF - 1 - f) for f in range(MF)]  # [512,256,128,64,32,16,8,4,2,1]
SMALL_D = [d for d in DIAG if d <= 64]         # [64,32,16,8,4,2,1]
F_OF_D = {DIAG[f]: f for f in range(MF)}
F128, F256, F512 = F_OF_D[128], F_OF_D[256], F_OF_D[512]

# per-table live-column window (start, width) in 256-wide table coords.
# S/Z are the pure-diagonal d=512 blocks (Delta -4/+4): single live col 128.
WOFF = {"C": 64, "P": 0, "M": 124, "S": 128, "Z": 128}
TW = {"C": 132, "P": 132, "M": 132, "S": 4, "Z": 4}
# vector-MAC singles: only the d=256 diagonal blocks (Delta -2/+2).
# For output block Jb the source block is Mb = Jb + delta; coeff column
# is s4[:, row, Mb].
SINGLES = [(0, -2), (1, 2)]

_CACHE = {}


def _build_program():
    import concourse.bacc as bacc
    import concourse.bass as bass
    import concourse.mybir as mybir
    from concourse import tile

    F32 = mybir.dt.float32
    BF16 = mybir.dt.bfloat16
    AX = mybir.AxisListType
    AF = mybir.ActivationFunctionType
    ALU = mybir.AluOpType

    nc = bacc.Bacc("TRN2", target_bir_lowering=False, debug=False)

    xb_d = nc.dram_tensor("xb", [BPC, SIZE], BF16, kind="ExternalInput").ap()
    lg_d = nc.dram_tensor("logit", [NT, MF], F32, kind="ExternalInput").ap()
    dg_d = nc.dram_tensor("dgs", [128, MF, NB], F32, kind="ExternalInput").ap()
    sb_d = nc.dram_tensor("sbc", [128, MF, NB], F32, kind="ExternalInput").ap()
    sp_d = nc.dram_tensor("spsh", [128, MF, NB], F32, kind="ExternalInput").ap()
    id_d = nc.dram_tensor("idstrip", [128, 384], BF16, kind="ExternalInput").ap()
    out_d = nc.dram_tensor("out", [BPC, SIZE], F32, kind="ExternalOutput").ap()
    pr_stage = nc.dram_tensor("pr_stage", [NT * MF], F32).ap()
    # DRAM staging for the shear tables (double buffered)
    stages = {
        (s, p): nc.dram_tensor(f"stg_{s}{p}", [128, NB * 256], BF16).ap()
        for s in "CPMSZ"
        for p in (0, 1)
    }

    def shear_src(s, p):
        """AP reading staged tables as dense banded blocks.

        block Mb, row m, col j  <-  stage[m, Mb*256 + 128 + j - m]
        flat offset = m*2048 + Mb*256 + 128 - m + j
        """
        flat = stages[(s, p)].rearrange("a b -> (a b)")
        return bass.AP(
            tensor=flat.tensor,
            offset=128,
            ap=[[NB * 256 - 1, 128], [256, NB], [1, 128]],
        )

    def mac_engine():
        # Pool has no ALU/PSUM path on TRN2; all MACs go to DVE
        return nc.vector

    with tile.TileContext(nc) as tc:
        with (
            tc.tile_pool(name="const", bufs=1) as cp,
            tc.tile_pool(name="T", bufs=1) as tp,
        ):
            # ---- load constants ----
            lg = cp.tile([NT, MF], F32, tag="lg")
            nc.sync.dma_start(lg[:, :], lg_d[:, :])
            dgs = cp.tile([128, MF, NB], F32, tag="dgs")
            nc.sync.dma_start(dgs[:, :, :], dg_d[:, :, :])
            sbc = cp.tile([128, MF, NB], F32, tag="sbc")
            nc.sync.dma_start(sbc[:, :, :], sb_d[:, :, :])
            spsh = cp.tile([128, MF, NB], F32, tag="spsh")
            nc.sync.dma_start(spsh[:, :, :], sp_d[:, :, :])
            idst = cp.tile([128, 384], BF16, tag="idst")
            nc.sync.dma_start(idst[:, :], id_d[:, :])

            # zero strip for stage-table padding (bf16 zeros)
            ztile = cp.tile([128, NB * 256], BF16, tag="ztile")
            nc.vector.memset(ztile[:, :], 0.0)
            for key in stages:
                nc.sync.dma_start(stages[key][:, :], ztile[:, :])

            # ---- softmax(logit) ----
            mx = cp.tile([NT, 1], F32, tag="mx")
            nc.vector.reduce_max(mx[:, :], lg[:, :], axis=AX.X)
            lgs = cp.tile([NT, MF], F32, tag="lgs")
            nc.vector.tensor_scalar_sub(lgs[:, :], lg[:, :], mx[:, :])
            ex = cp.tile([NT, MF], F32, tag="ex")
            nc.scalar.activation(ex[:, :], lgs[:, :], AF.Exp)
            sm = cp.tile([NT, 1], F32, tag="sm")
            nc.vector.reduce_sum(sm[:, :], ex[:, :], axis=AX.X)
            rs = cp.tile([NT, 1], F32, tag="rs")
            nc.vector.reciprocal(rs[:, :], sm[:, :])
            pr = cp.tile([NT, MF], F32, tag="pr")
            nc.vector.tensor_scalar_mul(pr[:, :], ex[:, :], rs[:, :])
            # broadcast probs to all 128 partitions (bounce via DRAM to get
            # a single-partition flat row first; partition_broadcast needs p0)
            nc.sync.dma_start(pr_stage.rearrange("(a b) -> a b", a=NT, b=MF), pr[:, :])
            prf = cp.tile([1, NT * MF], F32, tag="prf")
            nc.sync.dma_start(prf[:, :], pr_stage[None, :])
            pbc = cp.tile([128, NT, MF], F32, tag="pbc")
            nc.gpsimd.partition_broadcast(
                pbc[:, :, :].rearrange("p a b -> p (a b)"), prf[:, :]
            )

            # ---- T ping-pong buffers (bf16), T <- I ----
            Ta = [tp.tile([128, SIZE], BF16, tag=f"Ta{J}", name=f"Ta{J}") for J in range(NB)]
            Tb = [tp.tile([128, SIZE], BF16, tag=f"Tb{J}", name=f"Tb{J}") for J in range(NB)]
            for J in range(NB):
                nc.vector.memset(Ta[J][:, :], 0.0)
                nc.vector.tensor_copy(
                    Ta[J][:, 128 * J : 128 * J + 128], idst[:, 127:255]
                )

            # U = T^T, one contiguous tensor: U[p, k, n] = T[n, 128k+p]
            U = tp.tile([128, NB, SIZE], BF16, tag="U", name="U")

            # ---- compose: 10 steps of T <- A_i @ T ----
            with (
                tc.tile_pool(name="tab", bufs=1) as tbp,
                tc.tile_pool(name="lhs", bufs=1) as lp,
                tc.tile_pool(name="ctmp", bufs=2) as ctp,
                tc.tile_pool(name="cps", bufs=6, space="PSUM") as cps,
            ):
                # live-window tables in SBUF (fixed tiles, zero padding
                # written once; only live columns rewritten per step)
                tabs = {
                    (s, p): tbp.tile([128, NB, TW[s]], BF16, tag=f"tab{s}{p}", name=f"tab{s}{p}")
                    for s in "CPMSZ"
                    for p in (0, 1)
                }
                for t in tabs.values():
                    nc.vector.memset(t[:, :, :], 0.0)
                lhs = {
                    (s, p): lp.tile([128, NB, 128], BF16, tag=f"lhs{s}{p}", name=f"lhs{s}{p}")
                    for s in "CPMSZ"
                    for p in (0, 1)
                }
                s4s = {
                    p: lp.tile([128, 2, NB], F32, tag=f"s4_{p}", name=f"s4_{p}")
                    for p in (0, 1)
                }

                # PE warmup: keep the tensor engine busy while the first
                # tables build + stage, so it is at full p-state when the
                # real matmuls start.
                wps = cps.tile([128, 512], F32, tag="cacc")
                for w in range(10):
                    nc.tensor.matmul(
                        wps[:, :], idst[:, 127:255], Ta[0][:, 0:512],
                        start=(w == 0), stop=(w == 9),
                    )

                def build_tables(st, i):
                    """Vector-engine writes of step i's live table columns
                    + scaled singles coefficients (buffer p = st%2)."""
                    p = st % 2
                    tC, tP, tM = tabs[("C", p)], tabs[("P", p)], tabs[("M", p)]

                    def pcol(f):
                        return pbc[:, i, f : f + 1]

                    # D band: accumulate sum_f p_if * dg_f with fused MACs
                    dtmp = ctp.tile([128, NB], F32, tag="dtmp")
                    nc.vector.tensor_scalar_mul(dtmp[:, :], dgs[:, 0, :], pcol(0))
                    for f in range(1, MF - 1):
                        nc.vector.scalar_tensor_tensor(
                            dtmp[:, :], dgs[:, f, :], pcol(f), dtmp[:, :],
                            op0=ALU.mult, op1=ALU.add,
                        )
                    # last MAC writes straight into the C window (bf16)
                    nc.vector.scalar_tensor_tensor(
                        tC[:, :, 128 - WOFF["C"]], dgs[:, MF - 1, :], pcol(MF - 1),
                        dtmp[:, :], op0=ALU.mult, op1=ALU.add,
                    )

                    # banded columns (each: one op over 8 strided cols)
                    for n, d in enumerate(SMALL_D):
                        f = F_OF_D[d]
                        nc.vector.tensor_scalar_mul(
                            tC[:, :, 128 + d - WOFF["C"]], sbc[:, f, :], pcol(f))
                        nc.vector.tensor_scalar_mul(
                            tC[:, :, 128 - d - WOFF["C"]], spsh[:, f, :], pcol(f))
                        nc.vector.tensor_scalar_mul(
                            tP[:, :, d - WOFF["P"]], sbc[:, f, :], pcol(f))
                        nc.vector.tensor_scalar_mul(
                            tM[:, :, 256 - d - WOFF["M"]], spsh[:, f, :], pcol(f))
                    nc.vector.tensor_scalar_mul(
                        tP[:, :, 128 - WOFF["P"]], sbc[:, F128, :], pcol(F128))
                    nc.vector.tensor_scalar_mul(
                        tM[:, :, 128 - WOFF["M"]], spsh[:, F128, :], pcol(F128))

                    # d=512 pure-diagonal stationaries (tables S/Z, col 128)
                    tS, tZ = tabs[("S", p)], tabs[("Z", p)]
                    nc.scalar.activation(tS[:, :, 128 - WOFF["S"]], sbc[:, F512, :], AF.Copy, scale=pcol(F512))
                    nc.scalar.activation(tZ[:, :, 128 - WOFF["Z"]], spsh[:, F512, :], AF.Copy, scale=pcol(F512))

                    # d=256 singles coefficients, scaled by p_if
                    s4 = s4s[p]
                    nc.scalar.activation(s4[:, 0, :], sbc[:, F256, :], AF.Copy, scale=pcol(F256))
                    nc.scalar.activation(s4[:, 1, :], spsh[:, F256, :], AF.Copy, scale=pcol(F256))

                    # stage live windows + shear-read back as dense blocks
                    for s in "CPMSZ":
                        stg = stages[(s, p)].rearrange("a (b c) -> a b c", b=NB, c=256)
                        nc.sync.dma_start(
                            stg[:, :, WOFF[s] : WOFF[s] + TW[s]],
                            tabs[(s, p)][:, :, :],
                        )
                        nc.sync.dma_start(lhs[(s, p)][:, :, :], shear_src(s, p))

                # prefetch tables for the first two steps
                order = list(reversed(range(NT)))
                build_tables(0, order[0])
                build_tables(1, order[1])

                cur, nxt = Ta, Tb
                for st, i in enumerate(order):
                    p = st % 2
                    lC, lP, lM = lhs[("C", p)], lhs[("P", p)], lhs[("M", p)]
                    lS, lZ = lhs[("S", p)], lhs[("Z", p)]
                    s4 = s4s[p]

                    if st == 0:
                        # T = A_i @ I: everything restricted to 128-col
                        # aligned windows; column block Mb of nxt[Jb] is
                        # A_i's (Jb, Mb) block.
                        for Jb in range(NB):
                            live = []
                            for Mb in (Jb, Jb - 1, Jb + 1, Jb - 2, Jb + 2,
                                       Jb - 4, Jb + 4):
                                if 0 <= Mb < NB:
                                    live.append(Mb)
                            # PE handles d=512 diagonal blocks too
                            s1_extra = []
                            if Jb >= 4:
                                s1_extra.append((lS, Jb - 4))
                            if Jb <= NB - 5:
                                s1_extra.append((lZ, Jb + 4))
                            # zero the dead column blocks
                            for Mb in range(NB):
                                if Mb not in live:
                                    nc.vector.memset(
                                        nxt[Jb][:, 128 * Mb : 128 * Mb + 128], 0.0)
                            # PE blocks
                            pe_blocks = [(lC, Jb)]
                            if Jb >= 1:
                                pe_blocks.append((lP, Jb - 1))
                            if Jb <= NB - 2:
                                pe_blocks.append((lM, Jb + 1))
                            pe_blocks += s1_extra
                            for lt, Mb in pe_blocks:
                                lh = lt[:, Mb, :]
                                ps = cps.tile([128, 512], F32, tag="cacc")
                                nc.tensor.matmul(
                                    ps[:, 0:128], lh,
                                    cur[Mb][:, 128 * Mb : 128 * Mb + 128],
                                    start=True, stop=True,
                                )
                                nc.scalar.copy(
                                    nxt[Jb][:, 128 * Mb : 128 * Mb + 128],
                                    ps[:, 0:128])
                            # pure-diagonal blocks: coeff * I-block
                            for row, dl in SINGLES:
                                Mb = Jb + dl
                                if not (0 <= Mb < NB):
                                    continue
                                mac_engine().tensor_scalar_mul(
                                    nxt[Jb][:, 128 * Mb : 128 * Mb + 128],
                                    cur[Mb][:, 128 * Mb : 128 * Mb + 128],
                                    s4[:, row, Mb : Mb + 1],
                                )
                    else:
                        for Jb in range(NB):
                            mms = [(lC[:, Jb, :], Jb)]
                            if Jb >= 1:
                                mms.append((lP[:, Jb - 1, :], Jb - 1))
                            if Jb <= NB - 2:
                                mms.append((lM[:, Jb + 1, :], Jb + 1))
                            if Jb >= 4:
                                mms.append((lS[:, Jb - 4, :], Jb - 4))
                            if Jb <= NB - 5:
                                mms.append((lZ[:, Jb + 4, :], Jb + 4))
                            sgl = [
                                (row, Jb + dl) for row, dl in SINGLES
                                if 0 <= Jb + dl < NB
                            ]
                            for h in range(2):
                                hs = slice(512 * h, 512 * h + 512)
                                ps = cps.tile([128, 512], F32, tag="cacc")
                                for idx, (lh, Mb) in enumerate(mms):
                                    nc.tensor.matmul(
                                        ps[:, :], lh, cur[Mb][:, hs],
                                        start=(idx == 0),
                                        stop=(idx == len(mms) - 1),
                                    )
                                # PSUM partial -> bf16 accumulator in nxt
                                nc.scalar.copy(nxt[Jb][:, hs], ps[:, :])
                                # pure-diagonal MACs on top (in place)
                                for row, Mb in sgl:
                                    mac_engine().scalar_tensor_tensor(
                                        nxt[Jb][:, hs], cur[Mb][:, hs],
                                        s4[:, row, Mb : Mb + 1], nxt[Jb][:, hs],
                                        op0=ALU.mult, op1=ALU.add,
                                    )
                    if st + 2 < NT:
                        build_tables(st + 2, order[st + 2])
                    cur, nxt = nxt, cur

                # ---- U = T^T via DMA crossbar transposes ----
                T_fin = cur
                for Jb in range(NB):
                    nc.sync.dma_start(
                        U[:, :, 128 * Jb : 128 * Jb + 128],
                        T_fin[Jb][:, :],
                        transpose=True,
                    )

            # ---- batch phase ----
            with (
                tc.tile_pool(name="xt", bufs=16) as xtp,
                tc.tile_pool(name="op", bufs=6) as op,
                tc.tile_pool(name="ops", bufs=6, space="PSUM") as ops,
            ):
                # x arrives bf16; DMA-crossbar transpose straight into
                # stationary layout: xt[p, k, b] = x[b, 128k+p]
                xts = []
                for t in range(BPC // 128):
                    xt = xtp.tile([128, NB, 128], BF16, tag="xt")
                    nc.sync.dma_start(
                        xt[:, :, :],
                        xb_d[128 * t : 128 * t + 128, :],
                        transpose=True,
                    )
                    xts.append(xt)
                nout = [0]
                for h in range(2):
                    hs = slice(512 * h, 512 * h + 512)
                    for t in range(BPC // 128):
                        ps = ops.tile([128, 512], F32, tag="oacc")
                        for k in range(NB):
                            nc.tensor.matmul(
                                ps[:, :], xts[t][:, k, :], U[:, k, hs],
                                start=(k == 0), stop=(k == NB - 1),
                            )
                        ob = op.tile([128, 512], F32, tag="ob")
                        if nout[0] % 2 == 0:
                            nc.scalar.copy(ob[:, :], ps[:, :])
                        else:
                            nc.vector.tensor_copy(ob[:, :], ps[:, :])
                        nout[0] += 1
                        nc.sync.dma_start(
                            out_d[128 * t : 128 * t + 128, hs], ob[:, :])

    nc.compile()
    return nc


def _get_program():
    if "nc" not in _CACHE:
        _CACHE["nc"] = _build_program()
    return _CACHE["nc"]


LAST_RESULTS = {}


def kernel(input, diags, subdiags, superdiags, logit, _trace=False):
    import ml_dtypes
    from concourse.bass_utils import run_bass_kernel_spmd

    x = np.ascontiguousarray(np.asarray(input, dtype=np.float32))
    dg = np.asarray(diags, dtype=np.float32)
    sb = np.asarray(subdiags, dtype=np.float32)
    sp = np.asarray(superdiags, dtype=np.float32)
    lg = np.ascontiguousarray(np.asarray(logit, dtype=np.float32))

    xb = x.astype(ml_dtypes.bfloat16)

    # host staging (pure layout): zero unused tails, shift superdiags by d,
    # relayout coefficient vectors partition-major (m, f, block)
    sb_clean = np.zeros_like(sb)
    sp_shift = np.zeros_like(sp)
    for f in range(MF):
        d = DIAG[f]
        sb_clean[f, : SIZE - d] = sb[f, : SIZE - d]
        sp_shift[f, d:] = sp[f, : SIZE - d]

    def pm(v):  # (MF, SIZE) -> (128, MF, NB) with [m, f, blk] = v[f, 128*blk + m]
        return np.ascontiguousarray(
            v.reshape(MF, NB, 128).transpose(2, 0, 1)
        )

    dgs = pm(dg)
    sbc = pm(sb_clean)
    spsh = pm(sp_shift)
    idstrip = np.zeros((128, 384), dtype=np.float32)
    for m in range(128):
        idstrip[m, m + 127] = 1.0
    idstrip = idstrip.astype(ml_dtypes.bfloat16)

    nc = _get_program()
    in_maps = []
    for c in range(N_CORES):
        in_maps.append(
            {
                "xb": xb[BPC * c : BPC * (c + 1)],
                "logit": lg,
                "dgs": dgs,
                "sbc": sbc,
                "spsh": spsh,
                "idstrip": idstrip,
            }
        )
    res = run_bass_kernel_spmd(nc, in_maps, core_ids=list(range(N_CORES)), trace=_trace)
    LAST_RESULTS["res"] = res
    out = np.concatenate([res.results[c]["out"] for c in range(N_CORES)], axis=0)
    return out
